# revision 43
# baseline (speedup 1.0000x reference)
"""Trainium2 Bass kernel for a single transformer decoder layer.

Sharding: 8 cores = 4 batches x 2 head-groups (tensor parallel over heads for
attention; pairwise ReduceScatter; token-split FFN). All activations are kept
feature-major ("transposed") on device. v3: bf16 reduce-scatter payloads
(cast back to f32 during the SWDGE load), 2x-paced drain of the deferred
Wo/normalize steps so each RS fires ~40us earlier and fully overlaps the
attention tail, bf16 LayerNorm statistics (kills the fp32 PE matmuls),
triple-buffered W2 streaming, and JIT residual loads.
"""

import sys
from contextlib import ExitStack

for _p in ("/opt/trn_rl_repo",):
    if _p not in sys.path:
        sys.path.insert(0, _p)

import numpy as np

import concourse.bass as bass
import concourse.mybir as mybir
import concourse.tile as tile
from concourse import bacc
from concourse.bass_utils import run_bass_kernel_spmd

# ---- problem constants (hardcoded per spec) ----
B, S, D = 4, 2048, 1024
H, DK, DV, DFF = 16, 64, 64, 4096
EPS = 1e-5
SCALE = 1.0 / 32.0  # 1/sqrt(D)

NCORES = 8
HL = H // 2          # heads per core (local)
NP = HL // 2         # head-pairs per core (4)
TLOC = S // 2        # tokens owned per core after reduce-scatter (1024)
DC = D // 128        # d-model chunks (8)
FC = DFF // 128      # dff chunks (32)
FQ = FC // 4         # dff chunks per quarter (8)
QB = S // 512        # query blocks of 512 (4)

F32 = mybir.dt.float32
F32R = mybir.dt.float32r
BF16 = mybir.dt.bfloat16

_COMPILED = None


def _build():
    nc = bacc.Bacc("TRN2", target_bir_lowering=False, debug=False,
                   num_devices=NCORES)

    xT_d = nc.dram_tensor("xT", [D, S], BF16, kind="ExternalInput").ap()
    xTm_d = nc.dram_tensor("xTmine", [D, TLOC], F32, kind="ExternalInput").ap()
    wq_d = nc.dram_tensor("wq", [128, NP, DC, 128], BF16, kind="ExternalInput").ap()
    wk_d = nc.dram_tensor("wk", [128, NP, DC, 128], BF16, kind="ExternalInput").ap()
    wv_d = nc.dram_tensor("wv", [128, DC, 512], BF16, kind="ExternalInput").ap()
    wo_d = nc.dram_tensor("wo", [128, NP, DC, 128], BF16, kind="ExternalInput").ap()
    w1_d = nc.dram_tensor("w1", [4, 128, FQ, DC, 128], BF16, kind="ExternalInput").ap()
    w2_d = nc.dram_tensor("w2", [4, 128, 2, FC, 128], BF16, kind="ExternalInput").ap()
    b1_d = nc.dram_tensor("b1s", [128, FC], F32, kind="ExternalInput").ap()
    b2_d = nc.dram_tensor("b2s", [128, DC], F32, kind="ExternalInput").ap()
    g1_d = nc.dram_tensor("g1s", [128, DC], F32, kind="ExternalInput").ap()
    e1_d = nc.dram_tensor("e1s", [128, DC], F32, kind="ExternalInput").ap()
    g2_d = nc.dram_tensor("g2s", [128, DC], F32, kind="ExternalInput").ap()
    e2_d = nc.dram_tensor("e2s", [128, DC], F32, kind="ExternalInput").ap()
    mk_d = nc.dram_tensor("mask", [128, 4, 512], BF16, kind="ExternalInput").ap()

    outT_d = nc.dram_tensor("outT", [D, TLOC], F32, kind="ExternalOutput").ap()

    with tile.TileContext(nc) as tc:
        _emit(nc, tc, xT_d, xTm_d, wq_d, wk_d, wv_d, wo_d, w1_d, w2_d,
              b1_d, b2_d, g1_d, e1_d, g2_d, e2_d, mk_d, outT_d)
    nc.compile()
    return nc


def _emit(nc, tc, xT_d, xTm_d, wq_d, wk_d, wv_d, wo_d, w1_d, w2_d,
          b1_d, b2_d, g1_d, e1_d, g2_d, e2_d, mk_d, outT_d):
    AF = mybir.ActivationFunctionType
    ALU = mybir.AluOpType
    RG = [[0, 1], [2, 3], [4, 5], [6, 7]]

    es = ExitStack()
    with (
        es,
        tc.tile_pool(name="dram", bufs=1, space="DRAM") as dram,
        tc.tile_pool(name="const", bufs=1) as const,
    ):
        late = {}  # pools opened mid-emission via es (close at _emit end)
        MASK = const.tile([128, 4, 512], BF16)
        nc.scalar.dma_start(MASK[:], mk_d[:])
        onesf = const.tile([128, 1], F32)
        nc.vector.memset(onesf[:], 1.0)
        onesb = const.tile([128, 1], BF16)
        nc.vector.tensor_copy(onesb[:], onesf[:])
        onesbcf = const.tile([1, 128], F32)
        nc.vector.memset(onesbcf[:], 1.0)
        onesbc = const.tile([1, 128], F32R)
        nc.vector.tensor_copy(onesbc[:], onesbcf[:])
        onesbcb = const.tile([1, 128], BF16)
        nc.vector.tensor_copy(onesbcb[:], onesbcf[:])
        epst = const.tile([1, 1], F32)
        nc.vector.memset(epst[:], EPS)
        g1t = const.tile([128, DC], F32)
        e1t = const.tile([128, DC], F32)
        g2t = const.tile([128, DC], F32)
        e2t = const.tile([128, DC], F32)
        b1t = const.tile([128, FC], F32)
        b2t = const.tile([128, DC], F32)
        for t_, d_ in ((g1t, g1_d), (e1t, e1_d), (g2t, g2_d), (e2t, e2_d),
                       (b1t, b1_d), (b2t, b2_d)):
            nc.sync.dma_start(t_[:], d_[:])

        rs_in0 = dram.tile([2, D, 512], BF16)
        rs_in1 = dram.tile([2, D, 512], BF16)
        rs_out0 = dram.tile([D, 512], BF16)
        rs_out1 = dram.tile([D, 512], BF16)

        # ---- streamed FFN weights: instance i uses quarter i%4 ----
        w1q_tiles = []
        w2q_tiles = []

        def w1q_load(engine):
            i = len(w1q_tiles)
            t = late["pW1q"].tile([128, FQ, DC, 128], BF16, tag="w1",
                                  name=f"w1q_{i}")
            engine.dma_start(t[:], w1_d[i % 4])
            w1q_tiles.append(t)

        def w2q_load(engine):
            i = len(w2q_tiles)
            dout = i % 8
            t = late["pW2q"].tile([128, FC, 128], BF16, tag="w2",
                                  name=f"w2q_{i}")
            engine.dma_start(t[:], w2_d[dout // 2, :, dout % 2])
            w2q_tiles.append(t)

        # residual tiles (loads on gpsimd queue, ordered around collectives)
        res_tiles = {}

        def emit_xm_load(h):
            xm = late["pRES"].tile([128, DC, 512], F32, tag="XM", bufs=1,
                                   name=f"XMt{h}")
            nc.sync.dma_start(
                xm[:], xTm_d.rearrange("(dc p) t -> p dc t",
                                       p=128)[:, :, h * 512:(h + 1) * 512])
            res_tiles[("xm", h)] = xm

        def emit_res_loads(h, rso):
            # bf16 -> f32 cast happens inside the SWDGE DMA
            aor = late["pRES"].tile([128, DC, 512], F32R, tag="AOr", bufs=1,
                                    name=f"AOr{h}")
            nc.gpsimd.dma_start(
                aor[:], rso.rearrange("(dc p) t -> p dc t", p=128))
            res_tiles[h] = (aor, res_tiles[("xm", h)])

        # ============ attention ============
        with (
            tc.tile_pool(name="pQKT", bufs=1) as pQKT,
            tc.tile_pool(name="pV", bufs=1) as pV,
            tc.tile_pool(name="pWO", bufs=1) as pWO,
        ):
            QT = pQKT.tile([128, NP, S], BF16, tag="QT")          # 16KB
            KT = pQKT.tile([128, NP, S], BF16, tag="KT")          # 16KB
            V = pV.tile([128, S // 128, HL * 65], BF16, tag="V")  # 16.6KB
            nc.vector.tensor_copy(
                V[:].rearrange("p t (h c) -> p t h c", c=65)[:, :, :, 64:65],
                onesf[:, None, None, :].to_broadcast((128, S // 128, HL, 1)))
            wot = pWO.tile([128, NP, DC, 128], BF16, tag="wo")
            nc.scalar.dma_start(wot[:], wo_d[:])

            with (
                tc.tile_pool(name="pX", bufs=1) as pX,
                tc.tile_pool(name="pWQK", bufs=1) as pWQK,
            ):
                wqt = pWQK.tile([128, NP, DC, 128], BF16, tag="wq")
                wkt = pWQK.tile([128, NP, DC, 128], BF16, tag="wk")
                wvt = pWQK.tile([128, DC, 512], BF16, tag="wv")
                nc.sync.dma_start(wqt[:], wq_d[:])
                nc.sync.dma_start(wkt[:], wk_d[:])
                X = pX.tile([128, DC, S], BF16, tag="X")          # 32KB
                xT_r = xT_d.rearrange("(dc p) t -> p dc t", p=128)
                nc.sync.dma_start(X[:, 0:2, :], xT_r[:, 0:2, :])
                nc.sync.dma_start(wvt[:], wv_d[:])
                for hh in range(1, 4):
                    nc.sync.dma_start(X[:, 2 * hh:2 * hh + 2, :],
                                      xT_r[:, 2 * hh:2 * hh + 2, :])

                # Q/K projections, dc-outer so PE starts on the first X chunk
                with tc.tile_pool(name="psP", bufs=8, space="PSUM") as psP:
                    for p in range(NP):
                        pqs = [psP.tile([128, 512], F32, tag="proj",
                                        name=f"pq_{p}_{i}") for i in range(8)]
                        for dc in range(DC):
                            for tb in range(QB):
                                nc.tensor.matmul(
                                    pqs[tb][:], wqt[:, p, dc],
                                    X[:, dc, tb * 512:(tb + 1) * 512],
                                    start=(dc == 0), stop=(dc == DC - 1))
                            for tb in range(QB):
                                nc.tensor.matmul(
                                    pqs[4 + tb][:], wkt[:, p, dc],
                                    X[:, dc, tb * 512:(tb + 1) * 512],
                                    start=(dc == 0), stop=(dc == DC - 1))
                        for tb in range(QB):
                            tsl = slice(tb * 512, (tb + 1) * 512)
                            nc.vector.tensor_scalar_mul(QT[:, p, tsl],
                                                        pqs[tb][:], SCALE)
                            nc.vector.tensor_copy(KT[:, p, tsl], pqs[4 + tb][:])

                # V projection (needs all of X)
                with tc.tile_pool(name="psV", bufs=3, space="PSUM") as psV:
                    for tt in range(S // 128):
                        pv = psV.tile([128, 512], F32, tag="pv")
                        for dc in range(DC):
                            nc.tensor.matmul(pv[:],
                                             X[:, dc, tt * 128:(tt + 1) * 128],
                                             wvt[:, dc],
                                             start=(dc == 0), stop=(dc == DC - 1))
                        nc.vector.tensor_copy(
                            V[:, tt].rearrange("p (h c) -> p h c",
                                               c=65)[:, :, 0:64],
                            pv[:].rearrange("p (h c) -> p h c", c=64))

            # open the late pools now that X is freed; prefetch FFN weights
            late["pW1q"] = es.enter_context(
                tc.tile_pool(name="pW1q", bufs=2, side="right"))
            late["pW2q"] = es.enter_context(
                tc.tile_pool(name="pW2q", bufs=2, side="right"))
            late["pRES"] = es.enter_context(
                tc.tile_pool(name="pRES", bufs=1, side="right"))
            # dedicated region shared with no attention-phase pool: tiles
            # here carry no anti-deps against late-attention work, so the
            # gpsimd residual prep can run mid-attention
            late["pLN1"] = es.enter_context(
                tc.tile_pool(name="pLN1", bufs=1, side="right"))
            emit_xm_load(0)
            w1q_load(nc.sync)
            w1q_load(nc.sync)
            w2q_load(nc.sync)
            w2q_load(nc.sync)

            # ---- attention, qb-outer; normalize+Wo deferred into next qb ----
            with (
                tc.tile_pool(name="pCTX", bufs=2) as pCTX,
                tc.tile_pool(name="pE", bufs=4) as pE,
                tc.tile_pool(name="pAO", bufs=3) as pAO,
                tc.tile_pool(name="pDen", bufs=1) as pDen,
                tc.tile_pool(name="psS", bufs=2, space="PSUM") as psS,
                tc.tile_pool(name="psC", bufs=1, space="PSUM") as psC,
                tc.tile_pool(name="psW", bufs=2, space="PSUM") as psW,
            ):
                def make_finish_qb(qb, ctxbs, dents):
                    rsdst = rs_in0 if qb < 2 else rs_in1

                    def steps():
                        for p in range(NP):
                            rbA = psW.tile([64, 512], F32, tag="wo",
                                           name=f"rbA{qb}_{p}")
                            rbB = psW.tile([64, 512], F32, tag="wo",
                                           name=f"rbB{qb}_{p}")
                            nc.tensor.matmul(rbA[:], onesbcb[:, 0:64],
                                             dents[:, 2 * p],
                                             start=True, stop=True)
                            nc.tensor.matmul(rbB[:], onesbcb[:, 0:64],
                                             dents[:, 2 * p + 1],
                                             start=True, stop=True)
                            nc.vector.tensor_mul(ctxbs[0:64, p],
                                                 ctxbs[0:64, p], rbA[:])
                            nc.vector.tensor_mul(ctxbs[64:128, p],
                                                 ctxbs[64:128, p], rbB[:])
                            yield
                        for dout in range(DC):
                            po = psW.tile([128, 512], F32, tag="wo",
                                          name=f"po{qb}_{dout}")
                            for p in range(NP):
                                nc.tensor.matmul(po[:], wot[:, p, dout],
                                                 ctxbs[:, p],
                                                 start=(p == 0),
                                                 stop=(p == NP - 1))
                            ao = pAO.tile([128, 512], BF16, tag="ao")
                            nc.vector.tensor_copy(ao[:], po[:])
                            nc.sync.dma_start(
                                rsdst[qb % 2, dout * 128:(dout + 1) * 128, :],
                                ao[:])
                            yield
                        if qb == 1:
                            nc.gpsimd.collective_compute(
                                "ReduceScatter", mybir.AluOpType.add,
                                replica_groups=RG,
                                ins=[rs_in0.opt()], outs=[rs_out0.opt()])
                            # ordered on gpsimd right after RS-A: these fire
                            # the moment RS-A's output is available
                            emit_res_loads(0, rs_out0)
                        yield

                    return steps()

                pending = None
                for qb in range(QB):
                    qsl = slice(qb * 512, (qb + 1) * 512)
                    nkc = 4 * (qb + 1)
                    ctxbs = pCTX.tile([128, NP, 512], BF16, tag="ctxb",
                                      name=f"ctxb{qb}")
                    dents = pDen.tile([1, 8, 512], BF16, tag="dent",
                                      name=f"dent{qb}")
                    for p in range(NP):
                        ctxP = psC.tile([65, 2, 512], F32, tag="ctx",
                                        name=f"ctxP{qb}_{p}")

                        def emit_ctx(pkc, pe_):
                            st, sp = (pkc == 0), (pkc == nkc - 1)
                            nc.tensor.matmul(
                                ctxP[:, 0],
                                V[:, pkc, 2 * p * 65:(2 * p + 1) * 65],
                                pe_[:, 0], start=st, stop=sp)
                            nc.tensor.matmul(
                                ctxP[:, 1],
                                V[:, pkc, (2 * p + 1) * 65:(2 * p + 2) * 65],
                                pe_[:, 1], start=st, stop=sp)

                        inflight = []
                        for kc in range(nkc):
                            ksl = slice(kc * 128, (kc + 1) * 128)
                            sco = psS.tile([128, 2, 512], F32, tag="sc",
                                           name=f"s{qb}_{p}_{kc}")
                            nc.tensor.matmul(sco[:, 0], KT[0:64, p, ksl],
                                             QT[0:64, p, qsl],
                                             start=True, stop=True)
                            nc.tensor.matmul(sco[:, 1], KT[64:128, p, ksl],
                                             QT[64:128, p, qsl],
                                             start=True, stop=True)
                            # drain two deferred steps from the previous qb
                            # (front-loads Wo emission so the RS fires early)
                            if pending is not None and kc >= 1:
                                next(pending, None)
                                next(pending, None)
                            if len(inflight) >= 3:
                                emit_ctx(*inflight.pop(0))
                            et = pE.tile([128, 2, 512], BF16, tag="E")
                            nc.scalar.activation(et[:], sco[:], AF.Exp)
                            if kc >= 4 * qb:
                                mkc = kc - 4 * qb
                                nc.vector.tensor_mul(
                                    et[:], et[:],
                                    MASK[:, mkc:mkc + 1, :].to_broadcast(
                                        (128, 2, 512)))
                            inflight.append((kc, et))
                        for pr in inflight:
                            emit_ctx(*pr)
                        # unnormalized ctx + denominators out of PSUM
                        nc.vector.tensor_copy(ctxbs[0:64, p], ctxP[0:64, 0])
                        nc.vector.tensor_copy(ctxbs[64:128, p], ctxP[0:64, 1])
                        denf = pDen.tile([1, 2, 512], F32, tag="denf",
                                         bufs=1, name=f"denf{qb}_{p}")
                        nc.vector.tensor_copy(denf[:, 0], ctxP[64:65, 0])
                        nc.vector.tensor_copy(denf[:, 1], ctxP[64:65, 1])
                        nc.vector.reciprocal_approx_fast(denf[:], denf[:])
                        nc.vector.tensor_copy(
                            dents[:, 2 * p:2 * p + 2], denf[:])
                    if pending is not None:
                        for _ in pending:
                            pass
                    pending = make_finish_qb(qb, ctxbs, dents)
                for _ in pending:
                    pass

        # ======== post-attention: LN1 halves, FFN, LN2 ========
        with (
            tc.tile_pool(name="pFF", bufs=1) as pFF,
            tc.tile_pool(name="pO2", bufs=1) as pO2,
            tc.tile_pool(name="pH1", bufs=1) as pH1,
            tc.tile_pool(name="pLN", bufs=1) as pLN,
            tc.tile_pool(name="stDE", bufs=1) as stDE,
            tc.tile_pool(name="psF", bufs=2, space="PSUM") as psF,
            tc.tile_pool(name="psG", bufs=2, space="PSUM") as psG,
            tc.tile_pool(name="psD", bufs=1, space="PSUM") as psD,
            tc.tile_pool(name="psB", bufs=2, space="PSUM") as psB,
        ):
            pRES = late["pRES"]
            pLN1 = late["pLN1"]
            H1b = [pLN1.tile([128, DC, 512], BF16, tag=f"H1b{h}",
                             name=f"H1b{h}") for h in range(2)]
            FFt = pFF.tile([128, FC, 512], BF16, tag="FF")  # 32KB
            O2 = pO2.tile([128, DC, 512], BF16, tag="O2")   # 8KB

            def layer_norm(srcb, dst, gt, et, nm, pool, sq=None):
                """feature-major LN over features of [128, DC, 512].

                Both statistics and the normalize path read the bf16
                source `srcb` (the destinations are bf16 or feed bf16
                consumers, so fp32 sources would not buy accuracy).
                """
                if sq is None:
                    sq = pool.tile([128, DC, 512], BF16, tag="sqb",
                                   name=f"sq{nm}")
                    for dc in range(DC):
                        nc.scalar.activation(sq[:, dc], srcb[:, dc],
                                             AF.Square)
                pmu = psD.tile([1, 512], F32, tag="pmu", name=f"pm{nm}")
                psq = psD.tile([1, 512], F32, tag="psq", name=f"pq{nm}")
                for dc in range(DC):
                    nc.tensor.matmul(pmu[:], onesb[:], srcb[:, dc],
                                     start=(dc == 0), stop=(dc == DC - 1))
                for dc in range(DC):
                    nc.tensor.matmul(psq[:], onesb[:], sq[:, dc],
                                     start=(dc == 0), stop=(dc == DC - 1))
                mu_t = stDE.tile([1, 512], F32, tag="mu", name=f"mu{nm}")
                ex2_t = stDE.tile([1, 512], F32, tag="ex2", name=f"e2{nm}")
                var_t = stDE.tile([1, 512], F32, tag="var", name=f"va{nm}")
                srt_t = stDE.tile([1, 512], F32, tag="srt", name=f"sr{nm}")
                mu, ex2, var, srt = mu_t[:], ex2_t[:], var_t[:], srt_t[:]
                nc.vector.tensor_scalar_mul(mu, pmu[:], 1.0 / D)
                nc.vector.tensor_scalar_mul(ex2, psq[:], 1.0 / D)
                nc.vector.tensor_mul(var, mu, mu)
                nc.vector.tensor_sub(var, ex2, var)
                nc.scalar.activation(srt, var, AF.Sqrt, bias=epst[:])
                nmr_t = stDE.tile([1, 512], F32R, tag="nmr", name=f"nm{nm}")
                nmr = nmr_t[:]
                nc.vector.reciprocal_approx_fast(srt, srt)
                rstd = srt
                nc.vector.scalar_tensor_tensor(nmr, mu, -1.0, rstd,
                                               ALU.mult, ALU.mult)
                rstdb = psB.tile([128, 512], F32, tag="bc", name=f"rstdb{nm}")
                nmrb = psB.tile([128, 512], F32, tag="bc", name=f"nmrb{nm}")
                nc.tensor.matmul(rstdb[:], onesbcf[:], rstd,
                                 start=True, stop=True)
                nc.tensor.matmul(nmrb[:], onesbc[:], nmr,
                                 start=True, stop=True)
                xh = pLN.tile([128, DC, 512], F32R, tag="xh", name=f"xh{nm}")
                nc.vector.tensor_mul(
                    xh[:], srcb,
                    rstdb[:, None, :].to_broadcast((128, DC, 512)))
                nc.vector.tensor_add(
                    xh[:], xh[:], nmrb[:, None, :].to_broadcast((128, DC, 512)))
                for dc in range(DC):
                    nc.scalar.activation(dst[:, dc], xh[:, dc], AF.Identity,
                                         bias=et[:, dc:dc + 1],
                                         scale=gt[:, dc:dc + 1])

            def res_compute(h):
                # residual add + square run on the idle gpsimd engine into
                # pLN1 tiles, so they execute mid-attention (h=0) / mid-FFN
                # (h=1) the moment the RS output lands
                aor, xm = res_tiles[h]
                aorb = pLN1.tile([128, DC, 512], BF16, tag="aorb",
                                 name=f"aorb{h}")
                nc.gpsimd.tensor_add(aorb[:], aor[:], xm[:])
                sqg = pLN1.tile([128, DC, 512], BF16, tag="sqg",
                                name=f"sqg{h}")
                nc.gpsimd.tensor_mul(sqg[:], aorb[:], aorb[:])
                layer_norm(aorb[:], H1b[h][:], g1t, e1t, f"h{h}", pLN1,
                           sq=sqg)

            def ffn1_half(th):
                for fq in range(4):
                    w1q = w1q_tiles[th * 4 + fq]
                    for fi in range(FQ):
                        fc = fq * FQ + fi
                        pf = psF.tile([128, 512], F32, tag="ff")
                        for dc in range(DC):
                            nc.tensor.matmul(
                                pf[:], w1q[:, fi, dc], H1b[th][:, dc],
                                start=(dc == 0), stop=(dc == DC - 1))
                        nc.scalar.activation(FFt[:, fc], pf[:], AF.Relu,
                                             bias=b1t[:, fc:fc + 1])
                    if len(w1q_tiles) < 8:
                        w1q_load(nc.scalar)

            def ffn2_half(th, mid_cb=None):
                for dout in range(DC):
                    w2q = w2q_tiles[th * 8 + dout]
                    pg = psG.tile([128, 512], F32, tag="o2")
                    for fc in range(FC):
                        nc.tensor.matmul(pg[:], w2q[:, fc], FFt[:, fc],
                                         start=(fc == 0),
                                         stop=(fc == FC - 1))
                    nc.vector.scalar_tensor_tensor(
                        O2[:, dout], pg[:], b2t[:, dout:dout + 1],
                        H1b[th][:, dout], ALU.add, ALU.add)
                    if len(w2q_tiles) < 16:
                        w2q_load(nc.sync)
                    if dout == 5 and mid_cb is not None:
                        mid_cb()

            def finish_half(th):
                ot = pRES.tile([128, DC, 512], F32, tag="AOr", bufs=1,
                               name=f"ot{th}")
                layer_norm(O2[:], ot[:], g2t, e2t, f"o{th}", pLN)
                nc.sync.dma_start(
                    outT_d.rearrange(
                        "(dc p) t -> p dc t",
                        p=128)[:, :, th * 512:(th + 1) * 512],
                    ot[:])

            res_compute(0)
            emit_xm_load(1)             # sync: JIT residual for half 1
            nc.gpsimd.collective_compute(
                "ReduceScatter", mybir.AluOpType.add,
                replica_groups=RG,
                ins=[rs_in1.opt()], outs=[rs_out1.opt()])
            emit_res_loads(1, rs_out1)  # gpsimd: queued behind RS-B
            ffn1_half(0)
            ffn2_half(0, mid_cb=lambda: res_compute(1))
            finish_half(0)
            ffn1_half(1)
            ffn2_half(1)
            finish_half(1)


def _pack_inputs(x, Wq, Wk, Wv, Wo, ln1_g, ln1_b, W1, b1, W2, b2, ln2_g, ln2_b):
    """Build the 8 per-core input maps (all host-side numpy)."""
    import ml_dtypes
    bf = ml_dtypes.bfloat16
    f = np.float32
    x = np.asarray(x, f)
    Wq = np.asarray(Wq, f); Wk = np.asarray(Wk, f); Wv = np.asarray(Wv, f)
    Wo = np.asarray(Wo, f); W1 = np.asarray(W1, f); W2 = np.asarray(W2, f)
    in_maps = []
    # w1p[fq, pp, fi, dc, n] = W1[dc*128+pp, (fq*FQ+fi)*128+n]
    w1p = np.ascontiguousarray(
        W1.reshape(DC, 128, 4, FQ, 128).transpose(2, 1, 3, 0, 4)).astype(bf)
    # w2p[dq, pp, do, fc, n] = W2[fc*128+pp, (dq*2+do)*128+n]
    w2p = np.ascontiguousarray(
        W2.reshape(FC, 128, 4, 2, 128).transpose(2, 1, 3, 0, 4)).astype(bf)
    b1s = np.ascontiguousarray(np.asarray(b1, f).reshape(FC, 128).T)
    b2s = np.ascontiguousarray(np.asarray(b2, f).reshape(DC, 128).T)
    g1s = np.ascontiguousarray(np.asarray(ln1_g, f).reshape(DC, 128).T)
    e1s = np.ascontiguousarray(np.asarray(ln1_b, f).reshape(DC, 128).T)
    g2s = np.ascontiguousarray(np.asarray(ln2_g, f).reshape(DC, 128).T)
    e2s = np.ascontiguousarray(np.asarray(ln2_b, f).reshape(DC, 128).T)
    kk = np.arange(512)[:, None]
    qq = np.arange(512)[None, :]
    mask = (kk <= qq).astype(f).reshape(4, 128, 512).transpose(1, 0, 2)
    mask = np.ascontiguousarray(mask).astype(bf)

    for c in range(NCORES):
        b, j = c // 2, c % 2
        hb = j * HL
        xT = np.ascontiguousarray(x[b].T.astype(bf))
        xTm = np.ascontiguousarray(np.concatenate(
            [x[b, j * 512:(j + 1) * 512],
             x[b, (j + 2) * 512:(j + 3) * 512]]).T)
        wq = np.stack([np.concatenate([Wq[hb + 2 * p], Wq[hb + 2 * p + 1]], 1)
                       for p in range(NP)])  # [NP, D, 128]
        # device layout [128, NP, DC, 128]: wqd[pp, p, dc, n] = wq[p, dc*128+pp, n]
        wq = np.ascontiguousarray(
            wq.reshape(NP, DC, 128, 128).transpose(2, 0, 1, 3)).astype(bf)
        wk = np.stack([np.concatenate([Wk[hb + 2 * p], Wk[hb + 2 * p + 1]], 1)
                       for p in range(NP)])
        wk = np.ascontiguousarray(
            wk.reshape(NP, DC, 128, 128).transpose(2, 0, 1, 3)).astype(bf)
        wv = np.concatenate([Wv[hb + i] for i in range(HL)], 1)  # [D, 512]
        wv = np.ascontiguousarray(
            wv.reshape(DC, 128, 512).transpose(1, 0, 2)).astype(bf)
        wo = Wo[j * 512:(j + 1) * 512]  # [512, D]
        wo = np.ascontiguousarray(
            wo.reshape(NP, 128, DC, 128).transpose(1, 0, 2, 3)).astype(bf)
        in_maps.append({
            "xT": xT, "xTmine": xTm, "wq": wq, "wk": wk, "wv": wv, "wo": wo,
            "w1": w1p, "w2": w2p, "b1s": b1s, "b2s": b2s,
            "g1s": g1s, "e1s": e1s, "g2s": g2s, "e2s": e2s, "mask": mask,
        })
    return in_maps


def get_compiled():
    global _COMPILED
    if _COMPILED is None:
        _COMPILED = _build()
    return _COMPILED


def kernel(x, Wq, Wk, Wv, Wo, ln1_g, ln1_b, W1, b1, W2, b2, ln2_g, ln2_b,
           _trace=False):
    nc = get_compiled()
    in_maps = _pack_inputs(x, Wq, Wk, Wv, Wo, ln1_g, ln1_b, W1, b1, W2, b2,
                           ln2_g, ln2_b)
    res = run_bass_kernel_spmd(nc, in_maps, core_ids=list(range(NCORES)),
                               trace=_trace)
    out = np.zeros((B, S, D), np.float32)
    for c in range(NCORES):
        b, j = c // 2, c % 2
        o = res.results[c]["outT"]  # [D, TLOC]; cols = blocks {j, j+2}
        out[b, j * 512:(j + 1) * 512, :] = o[:, 0:512].T
        out[b, (j + 2) * 512:(j + 3) * 512, :] = o[:, 512:1024].T
    kernel.last_result = res
    return out

